# revision 6
# baseline (speedup 1.0000x reference)
"""DeepRNN (4-layer ReLU RNN, B=32 T=1024 H=512) on 8 trn2 NeuronCores.

Strategy: layer pipeline, 2 batch replicas x 4 stages. Each core owns one
layer; chunks of CH=32 timesteps flow down the pipeline via one 4-core
AllGather per iteration (cross-SEngine PAIR collectives hang under this
runtime, 4-core groups work). Uniform SPMD instruction stream; per-core
differences live in input data: weights, kappa reset schedule, one-hot
receive masks selecting the predecessor's gather entry.

Per-chunk compute on a stage core: the input projection (and bias, via a
K=1 ones-row matmul) accumulates into a PSUM-resident xw [P, JT, CH*BL];
the recurrence matmuls accumulate on top (start=False), so each timestep
needs exactly one fused Relu evacuation PSUM->SBUF (vector engine; gpsimd
is ~14x slower on these shapes and cannot read PSUM, scalar-engine ops
measured ~17us each here). The previous chunk's fc-head matmuls are
interleaved one-per-2-steps into the recurrence to fill PE bubbles.

Numerics: bf16 operands, fp32 PSUM accumulate (max rel err ~1e-2 vs fp32
reference, gate is 2e-2).
"""
import numpy as np
import ml_dtypes

import concourse.bass as bass
import concourse.bacc as bacc
import concourse.mybir as mybir
import concourse.tile as tile
from concourse.bass_utils import run_bass_kernel_spmd

# problem dims (hardcoded per contract)
B, T, H, O, L = 32, 1024, 512, 512, 4
P = 128
KT = JT = IT = OT = H // P          # 4 tiles per 512 dim
NCORES = 8
NREP = 2                            # batch replicas
BL = B // NREP                      # 16 batch lanes per core
CH = 32                             # timesteps per chunk
NCHUNK = T // CH                    # 32
DELAY = 2                           # stage-to-stage latency in iterations
NSLOT = 3                           # cc buffer parity depth
# +1: the fc head of chunk k runs interleaved into iteration k+1's recurrence
NITER = NCHUNK + DELAY * (L - 1) + 1  # 39
CB = CH * BL                        # 512 cols per chunk

BF = mybir.dt.bfloat16
F32 = mybir.dt.float32

# ONE 4-core AllGather per iteration serves every stage hop (cross-SEngine
# PAIR collectives hang under this runtime; 4-core groups work). Gather
# entry order = ascending group order = stage order, so stage r consumes
# entry r-1 via a per-core one-hot receive mask (stage 0 consumes nothing;
# entry 3 is consumed by no one).
GROUPS = [[0, 1, 2, 3], [4, 5, 6, 7]]
NRECV = 3                           # gather entries 0..2 are consumed


def _build_pl():
    nc = bacc.Bacc("TRN2", target_bir_lowering=False, debug=False,
                   num_devices=8)
    xin = nc.declare_dram_parameter("xin", [NCHUNK, P, IT, CB], BF, isOutput=False)
    wiT = nc.declare_dram_parameter("wiT", [P, IT * JT * P], BF, isOutput=False)
    whT = nc.declare_dram_parameter("whT", [P, KT * JT * P], BF, isOutput=False)
    wfcT = nc.declare_dram_parameter("wfcT", [P, KT * OT * P], BF, isOutput=False)
    biasT = nc.declare_dram_parameter("biasT", [1, JT * P], BF, isOutput=False)
    bfc = nc.declare_dram_parameter("bfc", [P, OT], F32, isOutput=False)
    kappa = nc.declare_dram_parameter("kappa", [P, NITER + 1], F32, isOutput=False)
    ridx = nc.declare_dram_parameter("ridx", [P, 1], mybir.dt.int32, isOutput=False)
    out = nc.declare_dram_parameter("out", [NCHUNK + 1, P, OT, CB], BF, isOutput=True)

    # separate tensors per parity slot so Tile's name-level DRAM dependency
    # tracking doesn't serialize iteration k's consume behind iteration
    # k-1's collective (they touch different parities)
    cc_in = [nc.dram_tensor(f"cc_in{s}", [P, JT * CB], BF)
             for s in range(NSLOT)]
    # entry 4 is never written: stage-0 cores' receive index points at it
    cc_out = [nc.dram_tensor(f"cc_out{s}", [5, P, JT * CB], BF)
              for s in range(NSLOT)]

    RELU = mybir.ActivationFunctionType.Relu
    COPY = mybir.ActivationFunctionType.Copy

    with tile.TileContext(nc) as tc:
        with (
            tc.tile_pool(name="const", bufs=1) as cpool,
            tc.tile_pool(name="state", bufs=1) as spool,
            tc.tile_pool(name="io2", bufs=2) as io2,
            tc.tile_pool(name="hsq", bufs=2) as hsq,
            tc.tile_pool(name="psX", bufs=1, space="PSUM") as psX,
            tc.tile_pool(name="psF", bufs=2, space="PSUM") as psF,
        ):
            wiT_sb = cpool.tile([P, IT * JT * P], BF)
            whT_sb = cpool.tile([P, KT * JT * P], BF)
            wfcT_sb = cpool.tile([P, KT * OT * P], BF)
            biasT_sb = cpool.tile([1, JT * P], BF)
            bfc_sb = cpool.tile([P, OT], F32)
            kappa_sb = cpool.tile([P, NITER + 1], F32)
            ridx_sb = cpool.tile([P, 1], mybir.dt.int32)
            nc.sync.dma_start(wiT_sb[:], wiT[:])
            nc.sync.dma_start(whT_sb[:], whT[:])
            nc.sync.dma_start(wfcT_sb[:], wfcT[:])
            nc.sync.dma_start(biasT_sb[:], biasT[:])
            nc.sync.dma_start(bfc_sb[:], bfc[:])
            nc.sync.dma_start(kappa_sb[:], kappa[:])
            nc.sync.dma_start(ridx_sb[:], ridx[:])

            ones_sb = cpool.tile([1, CB], BF)
            nc.vector.memset(ones_sb[:], 1.0)
            zrow = cpool.tile([P, JT * CB], BF)
            nc.vector.memset(zrow[:], 0.0)
            # zero the collective landing zones (consumed at iters 0/1
            # before any collective has produced data)
            for slot in range(NSLOT):
                for g in range(5):
                    nc.sync.dma_start(cc_out[slot][g], zrow[:])

            # persistent recurrent state
            hcur = spool.tile([P, KT, BL], BF)
            nc.vector.memset(hcur[:], 0.0)

            hprev = None                    # previous chunk's hseq (fc input)
            for k in range(NITER - 1):
                slot = k % NSLOT               # this iteration's gather slot
                rslot = (k - 1) % NSLOT        # consume the k-1 gather (DELAY=2)

                # ---- assemble input chunk: x + recv[pred entry] ----
                # one indirect DMA reads exactly the predecessor's gather
                # entry (stage-0 cores' index points at the always-zero
                # entry 4), so no receive masking is needed
                xb = io2.tile([P, IT, CB], BF, tag="xb")
                nc.sync.dma_start(xb[:], xin[min(k, NCHUNK - 1)])
                rg = io2.tile([P, IT * CB], BF, tag="rg")
                nc.gpsimd.indirect_dma_start(
                    out=rg[:],
                    out_offset=None,
                    in_=cc_out[rslot][:].rearrange("e p n -> (e p) n"),
                    in_offset=bass.IndirectOffsetOnAxis(ap=ridx_sb[:], axis=0),
                )
                insb = io2.tile([P, IT, CB], BF, tag="insb")
                nc.vector.tensor_tensor(
                    insb[:].rearrange("p i c -> p (i c)"),
                    xb[:].rearrange("p i c -> p (i c)"), rg[:],
                    mybir.AluOpType.add)

                # ---- send the PREVIOUS chunk to the successor. This sits
                # AFTER the receive: the collective post occupies the gpsimd
                # queue until the gather completes, so the indirect receive
                # must be queued ahead of it. Posted at the iteration head,
                # consumed at k+2: the gather gets ~2 iterations to run ----
                if hprev is not None and k <= NITER - 3:
                    nc.sync.dma_start(
                        cc_in[slot][:],
                        hprev[:].rearrange("p j c b -> p (j c b)"))
                    nc.gpsimd.collective_compute(
                        "AllGather", mybir.AluOpType.bypass, GROUPS,
                        ins=[cc_in[slot][:]], outs=[cc_out[slot][0:4]])

                # ---- input projection + bias into PSUM ----
                xps = psX.tile([P, JT, CB], F32, tag="xps")
                for jt in range(JT):
                    for it in range(IT):
                        nc.tensor.matmul(
                            xps[:, jt, :],
                            wiT_sb[:, (it * JT + jt) * P:(it * JT + jt + 1) * P],
                            insb[:, it, :],
                            start=(it == 0), stop=False, skip_group_check=True)
                    nc.tensor.matmul(
                        xps[:, jt, :],
                        biasT_sb[:, jt * P:(jt + 1) * P],
                        ones_sb[:],
                        start=False, stop=False, skip_group_check=True)

                # ---- recurrence over the chunk (accumulate onto psum xw),
                # with the PREVIOUS chunk's fc head matmuls interleaved into
                # the per-step PE bubbles (one fc matmul every 2 steps) ----
                hseq = hsq.tile([P, JT, CH, BL], BF, tag="hseq")
                osb = io2.tile([P, OT, CB], BF, tag="osb")
                fp = None
                for c in range(CH):
                    for jt in range(JT):
                        for kt in range(KT):
                            rhs = (hcur[:, kt, :] if c == 0
                                   else hseq[:, kt, c - 1, :])
                            nc.tensor.matmul(
                                xps[:, jt, c * BL:(c + 1) * BL],
                                whT_sb[:, (kt * JT + jt) * P:(kt * JT + jt + 1) * P],
                                rhs,
                                start=False, stop=(kt == KT - 1),
                                skip_group_check=True)
                    nc.vector.tensor_scalar_max(
                        hseq[:, :, c, :], xps[:, :, c * BL:(c + 1) * BL], 0.0)
                    if hprev is not None and c % 2 == 1 and c // 2 < 16:
                        ot, ht = divmod(c // 2, KT)
                        if ht == 0:
                            fp = psF.tile([P, CB], F32, tag="fp")
                        nc.tensor.matmul(
                            fp[:],
                            wfcT_sb[:, (ht * OT + ot) * P:(ht * OT + ot + 1) * P],
                            hprev[:, ht, :, :].rearrange("p c b -> p (c b)"),
                            start=(ht == 0), stop=(ht == KT - 1))
                        if ht == KT - 1:
                            nc.vector.tensor_scalar_add(
                                osb[:, ot, :], fp[:], bfc_sb[:, ot:ot + 1])
                # carry state into next iter, scaled by kappa[k+1]
                # (0 at a core's chunk-sequence start, 1 mid-sequence)
                nc.vector.tensor_scalar_mul(
                    hcur[:], hseq[:, :, CH - 1, :], kappa_sb[:, k + 1:k + 2])

                # ---- previous chunk's fc output (only stage-3 results kept) ----
                if hprev is not None:
                    oslice = (k - 1) - DELAY * (L - 1)
                    if not (0 <= oslice < NCHUNK):
                        oslice = NCHUNK
                    nc.sync.dma_start(out[oslice], osb[:])
                hprev = hseq

            # ---- epilogue: fc head of the final chunk ----
            osb_f = io2.tile([P, OT, CB], BF, tag="osb")
            for ot in range(OT):
                fp_f = psF.tile([P, CB], F32, tag="fp")
                for ht in range(KT):
                    nc.tensor.matmul(
                        fp_f[:],
                        wfcT_sb[:, (ht * OT + ot) * P:(ht * OT + ot + 1) * P],
                        hprev[:, ht, :, :].rearrange("p c b -> p (c b)"),
                        start=(ht == 0), stop=(ht == KT - 1))
                nc.vector.tensor_scalar_add(
                    osb_f[:, ot, :], fp_f[:], bfc_sb[:, ot:ot + 1])
            nc.sync.dma_start(out[NCHUNK - 1], osb_f[:])
    nc.compile()
    return nc


def _pack_w(w):
    """[512(out j), 512(in k)] -> [P, (kt*JT+jt)*P + j] = w[jt*P+j, kt*P+p]"""
    r = w.reshape(JT, P, KT, P)            # [jt, j, kt, p]
    r = r.transpose(3, 2, 0, 1)            # [p, kt, jt, j]
    return np.ascontiguousarray(r.reshape(P, KT * JT * P))


def _bcast(v):
    return np.ascontiguousarray(
        np.broadcast_to(v, (P,) + v.shape).astype(np.float32))


_NC_CACHE_PL = None
_last_in_maps = None
_last_nc = None


def _make_in_maps(x, W_ih, W_hh, b_ih, b_hh, W_fc, b_fc):
    in_maps = []
    wfcT = _pack_w(W_fc).astype(ml_dtypes.bfloat16)
    bfc_a = np.ascontiguousarray(b_fc.reshape(OT, P).T).astype(np.float32)
    for core in range(NCORES):
        s = core % 4          # pipeline stage == layer index
        rep = core // 4       # batch replica
        b0 = rep * BL

        if s == 0:
            xh = x[b0:b0 + BL]                                   # [BL, T, H]
            xr = xh.reshape(BL, NCHUNK, CH, IT, P)               # b k c i p
            xr = xr.transpose(1, 4, 3, 2, 0)                     # k p i c b
            xin = np.ascontiguousarray(
                xr.reshape(NCHUNK, P, IT, CB)).astype(ml_dtypes.bfloat16)
        else:
            xin = np.zeros((NCHUNK, P, IT, CB), ml_dtypes.bfloat16)

        wiT = _pack_w(W_ih[s]).astype(ml_dtypes.bfloat16)
        whT = _pack_w(W_hh[s]).astype(ml_dtypes.bfloat16)
        biasT = np.ascontiguousarray(
            (b_ih[s] + b_hh[s]).reshape(1, JT * P)).astype(ml_dtypes.bfloat16)

        # kappa[m] = 1 iff recurrent state carries INTO iteration m
        lo = DELAY * s
        kap = np.zeros(NITER + 1, np.float32)
        kap[lo + 1:lo + NCHUNK] = 1.0

        # stage s>0 consumes gather entry s-1 (its predecessor's send);
        # stage 0 points at the always-zero entry 4. Row-space index is
        # entry*P + p into cc_out viewed as [(5*P), JT*CB].
        e = (s - 1) if s > 0 else 4
        ridx_a = (e * P + np.arange(P, dtype=np.int32)).reshape(P, 1)

        in_maps.append({
            "xin": xin,
            "wiT": wiT, "whT": whT, "wfcT": wfcT,
            "biasT": biasT,
            "bfc": bfc_a,
            "kappa": _bcast(kap),
            "ridx": np.ascontiguousarray(ridx_a),
        })
    return in_maps


def _gather_out(results):
    y = np.empty((B, T, O), np.float32)
    for rep in range(NREP):
        arr = np.asarray(results[3 + 4 * rep]["out"])[:NCHUNK].astype(
            np.float32)                                  # [k, P, OT, CB]
        arr = arr.reshape(NCHUNK, P, OT, CH, BL)
        arr = arr.transpose(4, 0, 3, 2, 1)                   # b k c ot p
        y[rep * BL:(rep + 1) * BL] = arr.reshape(BL, T, O)
    return y


def _kernel_pl(x, W_ih, W_hh, b_ih, b_hh, W_fc, b_fc):
    global _NC_CACHE_PL, _last_in_maps, _last_nc
    in_maps = _make_in_maps(x, W_ih, W_hh, b_ih, b_hh, W_fc, b_fc)
    if _NC_CACHE_PL is None:
        _NC_CACHE_PL = _build_pl()
    _last_in_maps, _last_nc = in_maps, _NC_CACHE_PL
    res = run_bass_kernel_spmd(_NC_CACHE_PL, in_maps,
                               core_ids=list(range(NCORES)))
    return _gather_out(res.results)


def kernel(x, W_ih, W_hh, b_ih, b_hh, W_fc, b_fc):
    x = np.asarray(x); W_ih = np.asarray(W_ih); W_hh = np.asarray(W_hh)
    b_ih = np.asarray(b_ih); b_hh = np.asarray(b_hh)
    W_fc = np.asarray(W_fc); b_fc = np.asarray(b_fc)
    return _kernel_pl(x, W_ih, W_hh, b_ih, b_hh, W_fc, b_fc)



# revision 7
# speedup vs baseline: 1.0671x; 1.0671x over previous
"""DeepRNN (4-layer ReLU RNN, B=32 T=1024 H=512) on 8 trn2 NeuronCores.

Strategy: layer pipeline, 2 batch replicas x 4 stages. Each core owns one
layer; chunks of CH=32 timesteps flow down the pipeline via one 4-core
AllGather per iteration (cross-SEngine PAIR collectives hang under this
runtime, 4-core groups work). Uniform SPMD instruction stream; per-core
differences live in input data: weights, kappa reset schedule, one-hot
receive masks selecting the predecessor's gather entry.

Per-chunk compute on a stage core: the input projection (and bias, via a
K=1 ones-row matmul) accumulates into a PSUM-resident xw [P, JT, CH*BL];
the recurrence matmuls accumulate on top (start=False), so each timestep
needs exactly one fused Relu evacuation PSUM->SBUF (vector engine; gpsimd
is ~14x slower on these shapes and cannot read PSUM, scalar-engine ops
measured ~17us each here). The previous chunk's fc-head matmuls are
interleaved one-per-2-steps into the recurrence to fill PE bubbles.

Numerics: bf16 operands, fp32 PSUM accumulate (max rel err ~1e-2 vs fp32
reference, gate is 2e-2).
"""
import numpy as np
import ml_dtypes

import concourse.bass as bass
import concourse.bacc as bacc
import concourse.mybir as mybir
import concourse.tile as tile
from concourse.bass_utils import run_bass_kernel_spmd

# problem dims (hardcoded per contract)
B, T, H, O, L = 32, 1024, 512, 512, 4
P = 128
KT = JT = IT = OT = H // P          # 4 tiles per 512 dim
NCORES = 8
NREP = 2                            # batch replicas
BL = B // NREP                      # 16 batch lanes per core
CH = 32                             # timesteps per chunk
NCHUNK = T // CH                    # 32
DELAY = 3                           # stage-to-stage latency in iterations
NSLOT = 3                           # cc buffer parity depth
# +1: the fc head of chunk k runs interleaved into iteration k+1's recurrence
NITER = NCHUNK + DELAY * (L - 1) + 1  # 39
CB = CH * BL                        # 512 cols per chunk

BF = mybir.dt.bfloat16
F32 = mybir.dt.float32

# ONE 4-core AllGather per iteration serves every stage hop (cross-SEngine
# PAIR collectives hang under this runtime; 4-core groups work). Gather
# entry order = ascending group order = stage order, so stage r consumes
# entry r-1 via a per-core one-hot receive mask (stage 0 consumes nothing;
# entry 3 is consumed by no one).
GROUPS = [[0, 1, 2, 3], [4, 5, 6, 7]]
NRECV = 3                           # gather entries 0..2 are consumed


def _build_pl():
    nc = bacc.Bacc("TRN2", target_bir_lowering=False, debug=False,
                   num_devices=8)
    xin = nc.declare_dram_parameter("xin", [NCHUNK, P, IT, CB], BF, isOutput=False)
    wiT = nc.declare_dram_parameter("wiT", [P, IT * JT * P], BF, isOutput=False)
    whT = nc.declare_dram_parameter("whT", [P, KT * JT * P], BF, isOutput=False)
    wfcT = nc.declare_dram_parameter("wfcT", [P, KT * OT * P], BF, isOutput=False)
    biasT = nc.declare_dram_parameter("biasT", [1, JT * P], BF, isOutput=False)
    bfc = nc.declare_dram_parameter("bfc", [P, OT], F32, isOutput=False)
    kappa = nc.declare_dram_parameter("kappa", [P, NITER + 1], F32, isOutput=False)
    ridx = nc.declare_dram_parameter("ridx", [P, 1], mybir.dt.int32, isOutput=False)
    out = nc.declare_dram_parameter("out", [NCHUNK + 1, P, OT, CB], BF, isOutput=True)

    # separate tensors per parity slot so Tile's name-level DRAM dependency
    # tracking doesn't serialize iteration k's consume behind iteration
    # k-1's collective (they touch different parities)
    cc_in = [nc.dram_tensor(f"cc_in{s}", [P, JT * CB], BF)
             for s in range(NSLOT)]
    # entry 4 is never written: stage-0 cores' receive index points at it
    cc_out = [nc.dram_tensor(f"cc_out{s}", [5, P, JT * CB], BF)
              for s in range(NSLOT)]

    RELU = mybir.ActivationFunctionType.Relu
    COPY = mybir.ActivationFunctionType.Copy

    with tile.TileContext(nc) as tc:
        with (
            tc.tile_pool(name="const", bufs=1) as cpool,
            tc.tile_pool(name="state", bufs=1) as spool,
            tc.tile_pool(name="io2", bufs=2) as io2,
            tc.tile_pool(name="hsq", bufs=2) as hsq,
            tc.tile_pool(name="psX", bufs=1, space="PSUM") as psX,
            tc.tile_pool(name="psF", bufs=2, space="PSUM") as psF,
        ):
            wiT_sb = cpool.tile([P, IT * JT * P], BF)
            whT_sb = cpool.tile([P, KT * JT * P], BF)
            wfcT_sb = cpool.tile([P, KT * OT * P], BF)
            biasT_sb = cpool.tile([1, JT * P], BF)
            bfc_sb = cpool.tile([P, OT], F32)
            kappa_sb = cpool.tile([P, NITER + 1], F32)
            ridx_sb = cpool.tile([P, 1], mybir.dt.int32)
            nc.sync.dma_start(wiT_sb[:], wiT[:])
            nc.sync.dma_start(whT_sb[:], whT[:])
            nc.sync.dma_start(wfcT_sb[:], wfcT[:])
            nc.sync.dma_start(biasT_sb[:], biasT[:])
            nc.sync.dma_start(bfc_sb[:], bfc[:])
            nc.sync.dma_start(kappa_sb[:], kappa[:])
            nc.sync.dma_start(ridx_sb[:], ridx[:])

            ones_sb = cpool.tile([1, CB], BF)
            nc.vector.memset(ones_sb[:], 1.0)
            zrow = cpool.tile([P, JT * CB], BF)
            nc.vector.memset(zrow[:], 0.0)
            # zero the collective landing zones (consumed at iters 0/1
            # before any collective has produced data)
            for slot in range(NSLOT):
                for g in range(5):
                    nc.sync.dma_start(cc_out[slot][g], zrow[:])

            # persistent recurrent state
            hcur = spool.tile([P, KT, BL], BF)
            nc.vector.memset(hcur[:], 0.0)

            hprev = None                    # previous chunk's hseq (fc input)
            for k in range(NITER - 1):
                slot = k % NSLOT               # this iteration's gather slot
                rslot = (k - 2) % NSLOT        # consume the k-2 gather

                # ---- assemble input chunk: x + recv[pred entry] ----
                # one indirect DMA reads exactly the predecessor's gather
                # entry (stage-0 cores' index points at the always-zero
                # entry 4), so no receive masking is needed
                xb = io2.tile([P, IT, CB], BF, tag="xb")
                nc.sync.dma_start(xb[:], xin[min(k, NCHUNK - 1)])
                rg = io2.tile([P, IT * CB], BF, tag="rg")
                nc.gpsimd.indirect_dma_start(
                    out=rg[:],
                    out_offset=None,
                    in_=cc_out[rslot][:].rearrange("e p n -> (e p) n"),
                    in_offset=bass.IndirectOffsetOnAxis(ap=ridx_sb[:], axis=0),
                )
                insb = io2.tile([P, IT, CB], BF, tag="insb")
                nc.vector.tensor_tensor(
                    insb[:].rearrange("p i c -> p (i c)"),
                    xb[:].rearrange("p i c -> p (i c)"), rg[:],
                    mybir.AluOpType.add)

                # ---- send the PREVIOUS chunk to the successor. This sits
                # AFTER the receive: the collective post occupies the gpsimd
                # queue until the gather completes, so the indirect receive
                # must be queued ahead of it. Posted at the iteration head,
                # consumed at k+2: the gather gets ~2 iterations to run ----
                if hprev is not None and k <= NITER - 4:
                    nc.sync.dma_start(
                        cc_in[slot][:],
                        hprev[:].rearrange("p j c b -> p (j c b)"))
                    nc.gpsimd.collective_compute(
                        "AllGather", mybir.AluOpType.bypass, GROUPS,
                        ins=[cc_in[slot][:]], outs=[cc_out[slot][0:4]])

                # ---- input projection + bias into PSUM ----
                xps = psX.tile([P, JT, CB], F32, tag="xps")
                for jt in range(JT):
                    for it in range(IT):
                        nc.tensor.matmul(
                            xps[:, jt, :],
                            wiT_sb[:, (it * JT + jt) * P:(it * JT + jt + 1) * P],
                            insb[:, it, :],
                            start=(it == 0), stop=False, skip_group_check=True)
                    nc.tensor.matmul(
                        xps[:, jt, :],
                        biasT_sb[:, jt * P:(jt + 1) * P],
                        ones_sb[:],
                        start=False, stop=False, skip_group_check=True)

                # ---- recurrence over the chunk (accumulate onto psum xw),
                # with the PREVIOUS chunk's fc head matmuls interleaved into
                # the per-step PE bubbles (one fc matmul every 2 steps) ----
                hseq = hsq.tile([P, JT, CH, BL], BF, tag="hseq")
                osb = io2.tile([P, OT, CB], BF, tag="osb")
                fp = None
                for c in range(CH):
                    for jt in range(JT):
                        for kt in range(KT):
                            rhs = (hcur[:, kt, :] if c == 0
                                   else hseq[:, kt, c - 1, :])
                            nc.tensor.matmul(
                                xps[:, jt, c * BL:(c + 1) * BL],
                                whT_sb[:, (kt * JT + jt) * P:(kt * JT + jt + 1) * P],
                                rhs,
                                start=False, stop=(kt == KT - 1),
                                skip_group_check=True)
                    nc.vector.tensor_scalar_max(
                        hseq[:, :, c, :], xps[:, :, c * BL:(c + 1) * BL], 0.0)
                    if hprev is not None and c % 2 == 1 and c // 2 < 16:
                        ot, ht = divmod(c // 2, KT)
                        if ht == 0:
                            fp = psF.tile([P, CB], F32, tag="fp")
                        nc.tensor.matmul(
                            fp[:],
                            wfcT_sb[:, (ht * OT + ot) * P:(ht * OT + ot + 1) * P],
                            hprev[:, ht, :, :].rearrange("p c b -> p (c b)"),
                            start=(ht == 0), stop=(ht == KT - 1))
                        if ht == KT - 1:
                            nc.vector.tensor_scalar_add(
                                osb[:, ot, :], fp[:], bfc_sb[:, ot:ot + 1])
                # carry state into next iter, scaled by kappa[k+1]
                # (0 at a core's chunk-sequence start, 1 mid-sequence)
                nc.vector.tensor_scalar_mul(
                    hcur[:], hseq[:, :, CH - 1, :], kappa_sb[:, k + 1:k + 2])

                # ---- previous chunk's fc output (only stage-3 results kept) ----
                if hprev is not None:
                    oslice = (k - 1) - DELAY * (L - 1)
                    if not (0 <= oslice < NCHUNK):
                        oslice = NCHUNK
                    nc.sync.dma_start(out[oslice], osb[:])
                hprev = hseq

            # ---- epilogue: fc head of the final chunk ----
            osb_f = io2.tile([P, OT, CB], BF, tag="osb")
            for ot in range(OT):
                fp_f = psF.tile([P, CB], F32, tag="fp")
                for ht in range(KT):
                    nc.tensor.matmul(
                        fp_f[:],
                        wfcT_sb[:, (ht * OT + ot) * P:(ht * OT + ot + 1) * P],
                        hprev[:, ht, :, :].rearrange("p c b -> p (c b)"),
                        start=(ht == 0), stop=(ht == KT - 1))
                nc.vector.tensor_scalar_add(
                    osb_f[:, ot, :], fp_f[:], bfc_sb[:, ot:ot + 1])
            nc.sync.dma_start(out[NCHUNK - 1], osb_f[:])
    nc.compile()
    return nc


def _pack_w(w):
    """[512(out j), 512(in k)] -> [P, (kt*JT+jt)*P + j] = w[jt*P+j, kt*P+p]"""
    r = w.reshape(JT, P, KT, P)            # [jt, j, kt, p]
    r = r.transpose(3, 2, 0, 1)            # [p, kt, jt, j]
    return np.ascontiguousarray(r.reshape(P, KT * JT * P))


def _bcast(v):
    return np.ascontiguousarray(
        np.broadcast_to(v, (P,) + v.shape).astype(np.float32))


_NC_CACHE_PL = None
_last_in_maps = None
_last_nc = None


def _make_in_maps(x, W_ih, W_hh, b_ih, b_hh, W_fc, b_fc):
    in_maps = []
    wfcT = _pack_w(W_fc).astype(ml_dtypes.bfloat16)
    bfc_a = np.ascontiguousarray(b_fc.reshape(OT, P).T).astype(np.float32)
    for core in range(NCORES):
        s = core % 4          # pipeline stage == layer index
        rep = core // 4       # batch replica
        b0 = rep * BL

        if s == 0:
            xh = x[b0:b0 + BL]                                   # [BL, T, H]
            xr = xh.reshape(BL, NCHUNK, CH, IT, P)               # b k c i p
            xr = xr.transpose(1, 4, 3, 2, 0)                     # k p i c b
            xin = np.ascontiguousarray(
                xr.reshape(NCHUNK, P, IT, CB)).astype(ml_dtypes.bfloat16)
        else:
            xin = np.zeros((NCHUNK, P, IT, CB), ml_dtypes.bfloat16)

        wiT = _pack_w(W_ih[s]).astype(ml_dtypes.bfloat16)
        whT = _pack_w(W_hh[s]).astype(ml_dtypes.bfloat16)
        biasT = np.ascontiguousarray(
            (b_ih[s] + b_hh[s]).reshape(1, JT * P)).astype(ml_dtypes.bfloat16)

        # kappa[m] = 1 iff recurrent state carries INTO iteration m
        lo = DELAY * s
        kap = np.zeros(NITER + 1, np.float32)
        kap[lo + 1:lo + NCHUNK] = 1.0

        # stage s>0 consumes gather entry s-1 (its predecessor's send);
        # stage 0 points at the always-zero entry 4. Row-space index is
        # entry*P + p into cc_out viewed as [(5*P), JT*CB].
        e = (s - 1) if s > 0 else 4
        ridx_a = (e * P + np.arange(P, dtype=np.int32)).reshape(P, 1)

        in_maps.append({
            "xin": xin,
            "wiT": wiT, "whT": whT, "wfcT": wfcT,
            "biasT": biasT,
            "bfc": bfc_a,
            "kappa": _bcast(kap),
            "ridx": np.ascontiguousarray(ridx_a),
        })
    return in_maps


def _gather_out(results):
    y = np.empty((B, T, O), np.float32)
    for rep in range(NREP):
        arr = np.asarray(results[3 + 4 * rep]["out"])[:NCHUNK].astype(
            np.float32)                                  # [k, P, OT, CB]
        arr = arr.reshape(NCHUNK, P, OT, CH, BL)
        arr = arr.transpose(4, 0, 3, 2, 1)                   # b k c ot p
        y[rep * BL:(rep + 1) * BL] = arr.reshape(BL, T, O)
    return y


def _kernel_pl(x, W_ih, W_hh, b_ih, b_hh, W_fc, b_fc):
    global _NC_CACHE_PL, _last_in_maps, _last_nc
    in_maps = _make_in_maps(x, W_ih, W_hh, b_ih, b_hh, W_fc, b_fc)
    if _NC_CACHE_PL is None:
        _NC_CACHE_PL = _build_pl()
    _last_in_maps, _last_nc = in_maps, _NC_CACHE_PL
    res = run_bass_kernel_spmd(_NC_CACHE_PL, in_maps,
                               core_ids=list(range(NCORES)))
    return _gather_out(res.results)


def kernel(x, W_ih, W_hh, b_ih, b_hh, W_fc, b_fc):
    x = np.asarray(x); W_ih = np.asarray(W_ih); W_hh = np.asarray(W_hh)
    b_ih = np.asarray(b_ih); b_hh = np.asarray(b_hh)
    W_fc = np.asarray(W_fc); b_fc = np.asarray(b_fc)
    return _kernel_pl(x, W_ih, W_hh, b_ih, b_hh, W_fc, b_fc)



# revision 8
# speedup vs baseline: 1.0828x; 1.0147x over previous
"""DeepRNN (4-layer ReLU RNN, B=32 T=1024 H=512) on 8 trn2 NeuronCores.

Strategy: layer pipeline, 2 batch replicas x 4 stages. Each core owns one
layer; chunks of CH=32 timesteps flow down the pipeline via one 4-core
AllGather per iteration (cross-SEngine PAIR collectives hang under this
runtime, 4-core groups work). Uniform SPMD instruction stream; per-core
differences live in input data: weights, kappa reset schedule, one-hot
receive masks selecting the predecessor's gather entry.

Per-chunk compute on a stage core: the input projection (and bias, via a
K=1 ones-row matmul) accumulates into a PSUM-resident xw [P, JT, CH*BL];
the recurrence matmuls accumulate on top (start=False), so each timestep
needs exactly one fused Relu evacuation PSUM->SBUF (vector engine; gpsimd
is ~14x slower on these shapes and cannot read PSUM, scalar-engine ops
measured ~17us each here). The previous chunk's fc-head matmuls are
interleaved one-per-2-steps into the recurrence to fill PE bubbles.

Numerics: bf16 operands, fp32 PSUM accumulate (max rel err ~1e-2 vs fp32
reference, gate is 2e-2).
"""
import numpy as np
import ml_dtypes

import concourse.bass as bass
import concourse.bacc as bacc
import concourse.mybir as mybir
import concourse.tile as tile
from concourse.bass_utils import run_bass_kernel_spmd

# problem dims (hardcoded per contract)
B, T, H, O, L = 32, 1024, 512, 512, 4
P = 128
KT = JT = IT = OT = H // P          # 4 tiles per 512 dim
NCORES = 8
NREP = 2                            # batch replicas
BL = B // NREP                      # 16 batch lanes per core
CH = 32                             # timesteps per chunk
NCHUNK = T // CH                    # 32
DELAY = 3                           # stage-to-stage latency in iterations
NSLOT = 3                           # cc buffer parity depth
# +1: the fc head of chunk k runs interleaved into iteration k+1's recurrence
NITER = NCHUNK + DELAY * (L - 1) + 1  # 39
CB = CH * BL                        # 512 cols per chunk

BF = mybir.dt.bfloat16
F32 = mybir.dt.float32

# ONE 4-core AllGather per iteration serves every stage hop (cross-SEngine
# PAIR collectives hang under this runtime; 4-core groups work). Gather
# entry order = ascending group order = stage order, so stage r consumes
# entry r-1 via a per-core one-hot receive mask (stage 0 consumes nothing;
# entry 3 is consumed by no one).
GROUPS = [[0, 1, 2, 3], [4, 5, 6, 7]]
NRECV = 3                           # gather entries 0..2 are consumed


def _build_pl():
    nc = bacc.Bacc("TRN2", target_bir_lowering=False, debug=False,
                   num_devices=8)
    xin = nc.declare_dram_parameter("xin", [NCHUNK, P, IT, CB], BF, isOutput=False)
    wiT = nc.declare_dram_parameter("wiT", [P, IT * JT * P], BF, isOutput=False)
    whT = nc.declare_dram_parameter("whT", [P, KT * JT * P], BF, isOutput=False)
    wfcT = nc.declare_dram_parameter("wfcT", [P, KT * OT * P], BF, isOutput=False)
    biasT = nc.declare_dram_parameter("biasT", [1, JT * P], BF, isOutput=False)
    bfc = nc.declare_dram_parameter("bfc", [P, OT], F32, isOutput=False)
    kappa = nc.declare_dram_parameter("kappa", [P, NITER + 1], F32, isOutput=False)
    ridx = nc.declare_dram_parameter("ridx", [P, 1], mybir.dt.int32, isOutput=False)
    out = nc.declare_dram_parameter("out", [NCHUNK + 1, P, OT, CB], BF, isOutput=True)

    # separate tensors per parity slot so Tile's name-level DRAM dependency
    # tracking doesn't serialize iteration k's consume behind iteration
    # k-1's collective (they touch different parities)
    cc_in = [nc.dram_tensor(f"cc_in{s}", [P, JT * CB], BF)
             for s in range(NSLOT)]
    # entry 4 is never written: stage-0 cores' receive index points at it
    cc_out = [nc.dram_tensor(f"cc_out{s}", [5, P, JT * CB], BF)
              for s in range(NSLOT)]

    RELU = mybir.ActivationFunctionType.Relu
    COPY = mybir.ActivationFunctionType.Copy

    with tile.TileContext(nc) as tc:
        with (
            tc.tile_pool(name="const", bufs=1) as cpool,
            tc.tile_pool(name="state", bufs=1) as spool,
            tc.tile_pool(name="io2", bufs=2) as io2,
            tc.tile_pool(name="hsq", bufs=2) as hsq,
            tc.tile_pool(name="psX", bufs=1, space="PSUM") as psX,
            tc.tile_pool(name="psF", bufs=2, space="PSUM") as psF,
        ):
            wiT_sb = cpool.tile([P, IT * JT * P], BF)
            whT_sb = cpool.tile([P, KT * JT * P], BF)
            wfcT_sb = cpool.tile([P, KT * OT * P], BF)
            biasT_sb = cpool.tile([1, JT * P], BF)
            bfc_sb = cpool.tile([P, OT], F32)
            kappa_sb = cpool.tile([P, NITER + 1], F32)
            ridx_sb = cpool.tile([P, 1], mybir.dt.int32)
            nc.sync.dma_start(wiT_sb[:], wiT[:])
            nc.sync.dma_start(whT_sb[:], whT[:])
            nc.sync.dma_start(wfcT_sb[:], wfcT[:])
            nc.sync.dma_start(biasT_sb[:], biasT[:])
            nc.sync.dma_start(bfc_sb[:], bfc[:])
            nc.sync.dma_start(kappa_sb[:], kappa[:])
            nc.sync.dma_start(ridx_sb[:], ridx[:])

            ones_sb = cpool.tile([1, CB], BF)
            nc.vector.memset(ones_sb[:], 1.0)
            zrow = cpool.tile([P, JT * CB], BF)
            nc.vector.memset(zrow[:], 0.0)
            # zero the collective landing zones (consumed at iters 0/1
            # before any collective has produced data)
            for slot in range(NSLOT):
                for g in range(5):
                    nc.sync.dma_start(cc_out[slot][g], zrow[:])

            # persistent recurrent state
            hcur = spool.tile([P, KT, BL], BF)
            nc.vector.memset(hcur[:], 0.0)

            def emit_recv(k):
                # assemble iteration k's input chunk: x + recv[pred entry].
                # The gather consumed (posted at k-2) completes during k-1,
                # so when this is emitted at iteration k-1 (prefetch) the
                # receive runs mid-iteration and insb is ready at the head
                # of iteration k, off the critical path.
                xb = io2.tile([P, IT, CB], BF, tag="xb", name="xb")
                nc.sync.dma_start(xb[:], xin[min(k, NCHUNK - 1)])
                rg = io2.tile([P, IT * CB], BF, tag="rg", name="rg")
                nc.gpsimd.indirect_dma_start(
                    out=rg[:],
                    out_offset=None,
                    in_=cc_out[(k - 2) % NSLOT][:].rearrange("e p n -> (e p) n"),
                    in_offset=bass.IndirectOffsetOnAxis(ap=ridx_sb[:], axis=0),
                )
                insb = io2.tile([P, IT, CB], BF, tag="insb", name="insb")
                nc.vector.tensor_tensor(
                    insb[:].rearrange("p i c -> p (i c)"),
                    xb[:].rearrange("p i c -> p (i c)"), rg[:],
                    mybir.AluOpType.add)
                return insb

            hprev = None                    # previous chunk's hseq (fc input)
            insb_cur = None
            for k in range(NITER - 1):
                slot = k % NSLOT               # this iteration's gather slot

                if insb_cur is None:
                    insb_cur = emit_recv(k)
                insb = insb_cur

                # ---- send the PREVIOUS chunk to the successor. The
                # collective post occupies the gpsimd queue until the gather
                # completes, so receives for k+1 are queued after it and
                # start exactly when their data could exist. ----
                if hprev is not None and k <= NITER - 4:
                    nc.sync.dma_start(
                        cc_in[slot][:],
                        hprev[:].rearrange("p j c b -> p (j c b)"))
                    nc.gpsimd.collective_compute(
                        "AllGather", mybir.AluOpType.bypass, GROUPS,
                        ins=[cc_in[slot][:]], outs=[cc_out[slot][0:4]])

                # ---- prefetch next iteration's receive (its gather, posted
                # at k-1, completes during this iteration) ----
                insb_cur = emit_recv(k + 1) if k + 1 < NITER - 1 else None

                # ---- input projection + bias into PSUM ----
                xps = psX.tile([P, JT, CB], F32, tag="xps")
                for jt in range(JT):
                    for it in range(IT):
                        nc.tensor.matmul(
                            xps[:, jt, :],
                            wiT_sb[:, (it * JT + jt) * P:(it * JT + jt + 1) * P],
                            insb[:, it, :],
                            start=(it == 0), stop=False, skip_group_check=True)
                    nc.tensor.matmul(
                        xps[:, jt, :],
                        biasT_sb[:, jt * P:(jt + 1) * P],
                        ones_sb[:],
                        start=False, stop=False, skip_group_check=True)

                # ---- recurrence over the chunk (accumulate onto psum xw),
                # with the PREVIOUS chunk's fc head matmuls interleaved into
                # the per-step PE bubbles (one fc matmul every 2 steps) ----
                hseq = hsq.tile([P, JT, CH, BL], BF, tag="hseq")
                osb = io2.tile([P, OT, CB], BF, tag="osb")
                fp = None
                for c in range(CH):
                    for jt in range(JT):
                        for kt in range(KT):
                            rhs = (hcur[:, kt, :] if c == 0
                                   else hseq[:, kt, c - 1, :])
                            nc.tensor.matmul(
                                xps[:, jt, c * BL:(c + 1) * BL],
                                whT_sb[:, (kt * JT + jt) * P:(kt * JT + jt + 1) * P],
                                rhs,
                                start=False, stop=(kt == KT - 1),
                                skip_group_check=True)
                    nc.vector.tensor_scalar_max(
                        hseq[:, :, c, :], xps[:, :, c * BL:(c + 1) * BL], 0.0)
                    if hprev is not None and c % 2 == 1 and c // 2 < 16:
                        ot, ht = divmod(c // 2, KT)
                        if ht == 0:
                            fp = psF.tile([P, CB], F32, tag="fp")
                        nc.tensor.matmul(
                            fp[:],
                            wfcT_sb[:, (ht * OT + ot) * P:(ht * OT + ot + 1) * P],
                            hprev[:, ht, :, :].rearrange("p c b -> p (c b)"),
                            start=(ht == 0), stop=(ht == KT - 1))
                        if ht == KT - 1:
                            nc.vector.tensor_scalar_add(
                                osb[:, ot, :], fp[:], bfc_sb[:, ot:ot + 1])
                # carry state into next iter, scaled by kappa[k+1]
                # (0 at a core's chunk-sequence start, 1 mid-sequence)
                nc.vector.tensor_scalar_mul(
                    hcur[:], hseq[:, :, CH - 1, :], kappa_sb[:, k + 1:k + 2])

                # ---- previous chunk's fc output (only stage-3 results kept) ----
                if hprev is not None:
                    oslice = (k - 1) - DELAY * (L - 1)
                    if not (0 <= oslice < NCHUNK):
                        oslice = NCHUNK
                    nc.sync.dma_start(out[oslice], osb[:])
                hprev = hseq

            # ---- epilogue: fc head of the final chunk ----
            osb_f = io2.tile([P, OT, CB], BF, tag="osb")
            for ot in range(OT):
                fp_f = psF.tile([P, CB], F32, tag="fp")
                for ht in range(KT):
                    nc.tensor.matmul(
                        fp_f[:],
                        wfcT_sb[:, (ht * OT + ot) * P:(ht * OT + ot + 1) * P],
                        hprev[:, ht, :, :].rearrange("p c b -> p (c b)"),
                        start=(ht == 0), stop=(ht == KT - 1))
                nc.vector.tensor_scalar_add(
                    osb_f[:, ot, :], fp_f[:], bfc_sb[:, ot:ot + 1])
            nc.sync.dma_start(out[NCHUNK - 1], osb_f[:])
    nc.compile()
    return nc


def _pack_w(w):
    """[512(out j), 512(in k)] -> [P, (kt*JT+jt)*P + j] = w[jt*P+j, kt*P+p]"""
    r = w.reshape(JT, P, KT, P)            # [jt, j, kt, p]
    r = r.transpose(3, 2, 0, 1)            # [p, kt, jt, j]
    return np.ascontiguousarray(r.reshape(P, KT * JT * P))


def _bcast(v):
    return np.ascontiguousarray(
        np.broadcast_to(v, (P,) + v.shape).astype(np.float32))


_NC_CACHE_PL = None
_last_in_maps = None
_last_nc = None


def _make_in_maps(x, W_ih, W_hh, b_ih, b_hh, W_fc, b_fc):
    in_maps = []
    wfcT = _pack_w(W_fc).astype(ml_dtypes.bfloat16)
    bfc_a = np.ascontiguousarray(b_fc.reshape(OT, P).T).astype(np.float32)
    for core in range(NCORES):
        s = core % 4          # pipeline stage == layer index
        rep = core // 4       # batch replica
        b0 = rep * BL

        if s == 0:
            xh = x[b0:b0 + BL]                                   # [BL, T, H]
            xr = xh.reshape(BL, NCHUNK, CH, IT, P)               # b k c i p
            xr = xr.transpose(1, 4, 3, 2, 0)                     # k p i c b
            xin = np.ascontiguousarray(
                xr.reshape(NCHUNK, P, IT, CB)).astype(ml_dtypes.bfloat16)
        else:
            xin = np.zeros((NCHUNK, P, IT, CB), ml_dtypes.bfloat16)

        wiT = _pack_w(W_ih[s]).astype(ml_dtypes.bfloat16)
        whT = _pack_w(W_hh[s]).astype(ml_dtypes.bfloat16)
        biasT = np.ascontiguousarray(
            (b_ih[s] + b_hh[s]).reshape(1, JT * P)).astype(ml_dtypes.bfloat16)

        # kappa[m] = 1 iff recurrent state carries INTO iteration m
        lo = DELAY * s
        kap = np.zeros(NITER + 1, np.float32)
        kap[lo + 1:lo + NCHUNK] = 1.0

        # stage s>0 consumes gather entry s-1 (its predecessor's send);
        # stage 0 points at the always-zero entry 4. Row-space index is
        # entry*P + p into cc_out viewed as [(5*P), JT*CB].
        e = (s - 1) if s > 0 else 4
        ridx_a = (e * P + np.arange(P, dtype=np.int32)).reshape(P, 1)

        in_maps.append({
            "xin": xin,
            "wiT": wiT, "whT": whT, "wfcT": wfcT,
            "biasT": biasT,
            "bfc": bfc_a,
            "kappa": _bcast(kap),
            "ridx": np.ascontiguousarray(ridx_a),
        })
    return in_maps


def _gather_out(results):
    y = np.empty((B, T, O), np.float32)
    for rep in range(NREP):
        arr = np.asarray(results[3 + 4 * rep]["out"])[:NCHUNK].astype(
            np.float32)                                  # [k, P, OT, CB]
        arr = arr.reshape(NCHUNK, P, OT, CH, BL)
        arr = arr.transpose(4, 0, 3, 2, 1)                   # b k c ot p
        y[rep * BL:(rep + 1) * BL] = arr.reshape(BL, T, O)
    return y


def _kernel_pl(x, W_ih, W_hh, b_ih, b_hh, W_fc, b_fc):
    global _NC_CACHE_PL, _last_in_maps, _last_nc
    in_maps = _make_in_maps(x, W_ih, W_hh, b_ih, b_hh, W_fc, b_fc)
    if _NC_CACHE_PL is None:
        _NC_CACHE_PL = _build_pl()
    _last_in_maps, _last_nc = in_maps, _NC_CACHE_PL
    res = run_bass_kernel_spmd(_NC_CACHE_PL, in_maps,
                               core_ids=list(range(NCORES)))
    return _gather_out(res.results)


def kernel(x, W_ih, W_hh, b_ih, b_hh, W_fc, b_fc):
    x = np.asarray(x); W_ih = np.asarray(W_ih); W_hh = np.asarray(W_hh)
    b_ih = np.asarray(b_ih); b_hh = np.asarray(b_hh)
    W_fc = np.asarray(W_fc); b_fc = np.asarray(b_fc)
    return _kernel_pl(x, W_ih, W_hh, b_ih, b_hh, W_fc, b_fc)

